# revision 27
# baseline (speedup 1.0000x reference)
"""Trainium2 Bass kernel for the two-stream GQA attention problem.

Contract: kernel(**inputs) takes the FULL numpy inputs (as produced by
setup_inputs()) and returns the full outputs (out0, out1, k, v), matching
the reference jax implementation.

Distribution: 8 NeuronCores as a 4 (batch) x 2 (head-group) mesh.
Each core handles one batch element and 4 of the 8 query heads; the single
KV head is computed redundantly in each head-group.  The two partial
output projections per batch are summed on the host (the only cross-core
reduction).

Layout: the host pre-packs every streamed tensor into SBUF-shaped
[128, N] partition-major blocks with long contiguous DRAM lines (the DMA
trigger instruction costs ~5-17ns per descriptor line, so few-line
transfers with multi-KB lines are essential).  x is packed transposed
(x^T), after which every matmul consumes natural layouts only:
  Q^T[h,t]   = sum_w w_q[w,h] * x^T[w,t]          (lhsT=w_q tile, rhs=x^T)
  K^T[h,s]   likewise;  V[s,h] = sum_w x^T[w,s]*w_v[w,h] (lhsT=x^T, rhs=w_v)
  L^T[s,t]   = sum_h K^T[h,s] * Q^T[h,t]          (logits, transposed)
  P^T        = exp(L^T), causal-masked in place by a gpsimd
               affine_select on diagonal s-chunks (no max subtraction --
               logits here are O(5), exp cannot overflow, matches softmax)
  enc^T[h,t] = sum_s V[s,h] * P^T[s,t]
  sumbc[:,t] = sum_s P^T[s,t]  (all-ones [128,128] stationary: the column
               sums land broadcast across all 128 partitions, so the
               reciprocal runs on all DVE lanes)
  out[t,d]   = sum_{n,h} (enc^T[h,t]/sumbc[t]) * w_out[h,d]
RoPE is applied with DVE elementwise ops against host-precomputed
sin/cos tables laid out as [h_dim, t]; the 1/sqrt(HEAD_DIM) query scale is
folded into the q tables.  Causality is exploited by skipping fully-masked
s-chunks entirely and zeroing the diagonal chunks with affine_select.

All matmuls use float32r (full-rate fp32 PE path).  Outputs and constants
ride the gpsimd (SWDGE) queue so they never block the input stream on the
sync (HWDGE) queue.
"""

import numpy as np

HEAD_DIM = 256
NUM_HEADS = 8
MAX_WAVELENGTH = 10000.0

B, S0, S1 = 4, 768, 256
W0, W1 = 2048, 1024
T = S0 + S1  # 1024
NHC = 4      # heads per core
G = 2        # head groups
TT = 512     # attention t-tile
NTT = T // TT
SC = 128     # s-chunk
NSC = T // SC

_PROGRAM_CACHE = {}


# ----------------------------------------------------------------------------
# Workaround: this walrus build rejects instructions carrying more than one
# sync-wait command.  Split every multi-wait instruction into same-engine
# single-wait nops inserted right before it (engines run their stream in
# order, so semantics are identical).
# ----------------------------------------------------------------------------
def _split_multi_waits(nc, max_waits=1):
    import concourse.mybir as mybir

    for f in nc.m.functions:
        for bb in f.blocks:
            insts = bb.instructions
            i = 0
            while i < len(insts):
                inst = insts[i]
                si = getattr(inst, "sync_info", None)
                waits = list(si.on_wait) if si is not None and si.on_wait else []
                if len(waits) > max_waits:
                    si.on_wait = waits[:max_waits]
                    rest = waits[max_waits:]
                    eng = nc.engines[inst.engine]
                    for j in range(0, len(rest), max_waits):
                        chunk = rest[j : j + max_waits]
                        bi = eng.nop()
                        nop_inst = bi.ins
                        src = nc.cur_bb.bb.instructions
                        assert src[-1] is nop_inst
                        src.pop()
                        nsi = nop_inst.sync_info
                        if nsi is None:
                            nop_inst.sync_info = mybir.SyncInfo(
                                on_wait=chunk, on_update=[]
                            )
                        else:
                            nsi.on_wait = chunk
                        insts.insert(i, nop_inst)
                        i += 1
                i += 1


def _pack(mat):
    """[nw*128, H] -> [128, nw*H] partition-major (row r of tile i at
    partition r, columns i*H..)."""
    nw = mat.shape[0] // 128
    return np.ascontiguousarray(
        mat.reshape(nw, 128, -1).transpose(1, 0, 2).reshape(128, -1)
    )


def _build_program():
    import concourse.bass as bass
    import concourse.mybir as mybir
    import concourse.tile as tile
    import concourse.tile_utils as tile_utils

    # stale 192KB cap leaves 16KB/partition unused on trn2 (208KB usable)
    if getattr(tile_utils, "max_sbuf_usage", 0) < 204 * 1024:
        tile_utils.max_sbuf_usage = 204 * 1024

    F32 = mybir.dt.float32
    F32R = mybir.dt.float32r
    EXP = mybir.ActivationFunctionType.Exp
    LN = mybir.ActivationFunctionType.Ln

    nc = bass.Bass("TRN2", target_bir_lowering=False, debug=False)

    def din(name, shape, dt=F32R):
        return nc.dram_tensor(name, list(shape), dt, kind="ExternalInput").ap()

    # all inputs pre-packed to [128, N] partition-major on the host
    xp0 = din("xp0", [128, 16 * S0])
    xp1 = din("xp1", [128, 8 * S1])
    wq0 = din("wq0", [NHC, 128, 16 * HEAD_DIM])
    wq1 = din("wq1", [NHC, 128, 8 * HEAD_DIM])
    wk0 = din("wk0", [128, 16 * HEAD_DIM])
    wk1 = din("wk1", [128, 8 * HEAD_DIM])
    wv0 = din("wv0", [128, 16 * HEAD_DIM])
    wv1 = din("wv1", [128, 8 * HEAD_DIM])
    wo0 = din("wo0", [NHC, HEAD_DIM, W0])
    wo1 = din("wo1", [NHC, HEAD_DIM, W1])
    # cosq | sinq | cosk | sink concatenated along the free axis
    tbls = din("tbls", [128, 4 * T], F32)
    onesd = din("onesd", [128, 128])

    out0p = nc.dram_tensor("out0p", [S0, W0], F32, kind="ExternalOutput").ap()
    out1p = nc.dram_tensor("out1p", [S1, W1], F32, kind="ExternalOutput").ap()
    kT_d = nc.dram_tensor("kT", [HEAD_DIM, T], F32R, kind="ExternalOutput").ap()
    v_d = nc.dram_tensor("v", [T, HEAD_DIM], F32R, kind="ExternalOutput").ap()

    # token segments: (t0, length, stream); stream 0 -> xp0, stream 1 -> xp1
    SEGS = [(0, 512, 0), (512, 256, 0), (768, 256, 1)]
    NW = {0: W0 // 128, 1: W1 // 128}

    import contextlib

    with tile.TileContext(nc) as tc, \
         nc.allow_low_precision("float32r is a rounded fp32 for the PE fast path"):
        with contextlib.ExitStack() as ctx:
            persist = ctx.enter_context(tc.tile_pool(name="persist", bufs=1))

            # persistent activation tensors (encT lives in a later scope)
            QT = [
                [persist.tile([128, T], F32R, name=f"QT{n}_{c}", tag=f"QT{n}_{c}")
                 for c in range(2)]
                for n in range(NHC)
            ]
            KT = [persist.tile([128, T], F32R, name=f"KT{c}", tag=f"KT{c}")
                  for c in range(2)]
            V = [persist.tile([128, HEAD_DIM], F32R, name=f"V{s}", tag=f"V{s}")
                 for s in range(NSC)]

            # constants (DMAs are emitted inside phase 1, after the
            # startup-critical weight loads, on the gpsimd/SWDGE queue)
            ttbl = persist.tile([128, 4 * T], F32, name="ttbl", tag="ttbl")
            tcosq = ttbl[:, 0 * T : 1 * T]
            tsinq = ttbl[:, 1 * T : 2 * T]
            tcosk = ttbl[:, 2 * T : 3 * T]
            tsink = ttbl[:, 3 * T : 4 * T]
            ones_t = persist.tile([128, 128], F32R, name="ones_t", tag="ones_t")

            # ---------------- phase 1: projections -----------------------
            def rope_muls(qp0, qp1, cos_t, sin_t, t0, ln, tmp_pool):
                # DVE multiplies (psum-reading); qp0 reads first so its psum
                # bank frees early.  The add/sub combines run on gpsimd.
                cs = cos_t[:, t0 : t0 + ln]
                sn = sin_t[:, t0 : t0 + ln]
                a = tmp_pool.tile([128, 512], F32, name="rope_a", tag="rope_a")
                nc.vector.tensor_mul(a[:, :ln], qp0[:], cs)
                b2 = tmp_pool.tile([128, 512], F32, name="rope_b2", tag="rope_b2")
                nc.vector.tensor_mul(b2[:, :ln], qp0[:], sn)
                b = tmp_pool.tile([128, 512], F32, name="rope_b", tag="rope_b")
                nc.vector.tensor_mul(b[:, :ln], qp1[:], sn)
                a2 = tmp_pool.tile([128, 512], F32, name="rope_a2", tag="rope_a2")
                nc.vector.tensor_mul(a2[:, :ln], qp1[:], cs)
                return a, b, a2, b2

            def rope_combine(parts, dst0, dst1, t0, ln):
                a, b, a2, b2 = parts
                nc.gpsimd.tensor_sub(dst0[:, t0 : t0 + ln], a[:, :ln], b[:, :ln])
                nc.gpsimd.tensor_add(dst1[:, t0 : t0 + ln], a2[:, :ln], b2[:, :ln])

            def load_packed(src, ncols, nsplit, wpool, tag, eng=None):
                """Load a packed [128, ncols] DRAM tensor in nsplit 2D DMAs;
                returns the list of SBUF tiles (each [128, ncols/nsplit])."""
                step = ncols // nsplit
                out = []
                for bi in range(nsplit):
                    t = wpool.tile([128, step], F32R, name=tag, tag=tag)
                    (eng or nc.sync).dma_start(
                        t[:], src[:, bi * step : (bi + 1) * step]
                    )
                    out.append(t)
                return out

            def qk_proj(w0_sl, w1_sl, cos_t, sin_t, dst, qpsum, tmp_pool,
                        pset=0):
                # psum tags: A{c} [512] / B{c} [256]; two alternating tag sets
                # so consecutive heads never wait on each other's ropes.
                for stream, w_sl in ((0, w0_sl), (1, w1_sl)):
                    segs = [s for s in SEGS if s[2] == stream]
                    ps = {}
                    for c in range(2):
                        for (t0, ln, _s) in segs:
                            tag = (f"qk_psA{c}_{pset}" if ln == 512 else
                                   f"qk_psB{c}_{pset}")
                            ps[(c, t0)] = qpsum.tile([128, ln], F32, name=tag,
                                                     tag=tag)
                    nw = NW[stream]
                    xt = xt0 if stream == 0 else xt1
                    for i in range(nw):
                        wt = w_sl(i)
                        for c in range(2):
                            lhs = wt[:, c * 128 : (c + 1) * 128]
                            for (t0, ln, _s) in segs:
                                rel0 = t0 if stream == 0 else t0 - S0
                                nc.tensor.matmul(
                                    ps[(c, t0)][:],
                                    lhs,
                                    xt[i](rel0, ln),
                                    start=(i == 0),
                                    stop=(i == nw - 1),
                                )
                    parts = {}
                    for (t0, ln, _s) in segs:
                        parts[t0] = rope_muls(ps[(0, t0)], ps[(1, t0)],
                                              cos_t, sin_t, t0, ln, tmp_pool)
                    for (t0, ln, _s) in segs:
                        rope_combine(parts[t0], dst[0], dst[1], t0, ln)

            with tc.tile_pool(name="xt", bufs=1) as xtp, \
                 tc.tile_pool(name="wtiles", bufs=6) as wpool, \
                 tc.tile_pool(name="ropetmp", bufs=3) as tmp_pool:
                # startup-ordered loads: first K weights and the x batches
                # they pair with, interleaved on the sync queue; wk1 and the
                # constants ride the gpsimd queue in parallel.
                wk0b = []
                wk0b.append(wpool.tile([128, 8 * HEAD_DIM], F32R, name="wb",
                                       tag="wb"))
                nc.sync.dma_start(wk0b[0][:], wk0[:, : 8 * HEAD_DIM])
                xtb0 = []

                def _load_x0(bi):
                    t = xtp.tile([128, 2 * S0], F32R, name=f"xtb0_{bi}",
                                 tag=f"xtb0_{bi}")
                    nc.sync.dma_start(
                        t[:], xp0[:, bi * 2 * S0 : (bi + 1) * 2 * S0]
                    )
                    xtb0.append(t)

                for bi in range(3):
                    _load_x0(bi)
                wk0b.append(wpool.tile([128, 8 * HEAD_DIM], F32R, name="wb",
                                       tag="wb"))
                nc.sync.dma_start(wk0b[1][:], wk0[:, 8 * HEAD_DIM :])
                for bi in range(3, 8):
                    _load_x0(bi)
                xtb1 = xtp.tile([128, 8 * S1], F32R, name="xtb1", tag="xtb1")
                nc.sync.dma_start(xtb1[:], xp1[:])
                nc.gpsimd.dma_start(ones_t[:], onesd[:])
                wk1b = wpool.tile([128, 8 * HEAD_DIM], F32R, name="wb", tag="wb")
                nc.gpsimd.dma_start(wk1b[:], wk1[:])
                nc.gpsimd.dma_start(ttbl[:], tbls[:])

                # HAM warmup: dummy matmuls keep the PE clock at 8/8 while the
                # initial DMA stream lands (PE has nothing real to do yet).
                with tc.tile_pool(name="warm", bufs=1, space="PSUM") as warmp:
                    wps = warmp.tile([128, 128], F32, name="warm_ps",
                                     tag="warm_ps")
                    for wi in range(60):
                        nc.tensor.matmul(wps[:], ones_t[:], ones_t[:],
                                         start=(wi % 10 == 0),
                                         stop=(wi % 10 == 9))

                def _xt0(i):
                    base = (i % 2) * S0
                    return lambda rel0, ln: xtb0[i // 2][:, base + rel0 :
                                                         base + rel0 + ln]

                def _xt1(i):
                    base = (i % 8) * S1
                    return lambda rel0, ln: xtb1[:, base + rel0 : base + rel0 + ln]

                xt0 = [_xt0(i) for i in range(NW[0])]
                xt1 = [_xt1(i) for i in range(NW[1])]

                # K projection + rope + output
                with tc.tile_pool(name="qkpsum", bufs=1, space="PSUM") as qpsum:
                    qk_proj(
                        lambda i: wk0b[i // 8][:, (i % 8) * HEAD_DIM :
                                               (i % 8 + 1) * HEAD_DIM],
                        lambda i: wk1b[:, i * HEAD_DIM : (i + 1) * HEAD_DIM],
                        tcosk, tsink, KT, qpsum, tmp_pool, pset=0)
                for c in range(2):
                    nc.gpsimd.dma_start(kT_d[c * 128 : (c + 1) * 128, :], KT[c][:])

                # V projection (natural layout), all 6+2 chunks in one pass
                with tc.tile_pool(name="vpsum", bufs=1, space="PSUM") as vpsum:
                    for stream, wv_d in ((0, wv0), (1, wv1)):
                        nw = NW[stream]
                        xt = xt0 if stream == 0 else xt1
                        scs = range(0, 6) if stream == 0 else range(6, 8)
                        vps = {
                            sc: vpsum.tile([128, HEAD_DIM], F32,
                                           name=f"v_ps{sc % 6}", tag=f"v_ps{sc % 6}")
                            for sc in scs
                        }
                        wb = load_packed(wv_d, nw * HEAD_DIM, max(1, nw // 8),
                                         wpool, "wb")
                        for i in range(nw):
                            wt = wb[i // 8][:, (i % 8) * HEAD_DIM :
                                            (i % 8) * HEAD_DIM + HEAD_DIM]
                            for sc in scs:
                                rel0 = sc * 128 if stream == 0 else sc * 128 - S0
                                nc.tensor.matmul(
                                    vps[sc][:],
                                    xt[i](rel0, 128),
                                    wt[:],
                                    start=(i == 0),
                                    stop=(i == nw - 1),
                                )
                        for sc in scs:
                            nc.scalar.copy(V[sc][:], vps[sc][:])
                            nc.gpsimd.dma_start(
                                v_d[sc * 128 : (sc + 1) * 128, :], V[sc][:]
                            )

                # Q projections
                with tc.tile_pool(name="qpsum", bufs=1, space="PSUM") as qpsum:
                    for n in range(NHC):
                        wb0 = load_packed(wq0[n], 16 * HEAD_DIM, 2, wpool, "wb")
                        wb1 = load_packed(wq1[n], 8 * HEAD_DIM, 1, wpool, "wb")

                        def _slb(wb):
                            return lambda i: wb[i // 8][
                                :, (i % 8) * HEAD_DIM : (i % 8 + 1) * HEAD_DIM
                            ]

                        qk_proj(_slb(wb0), _slb(wb1), tcosq, tsinq, QT[n],
                                qpsum, tmp_pool, pset=n % 2)

            # ---------------- phases 2+3 ----------------------------------
            # encT/wo/stage pools open BEFORE the attention pools so their
            # SBUF region is disjoint from the attention tiles: the wo DMA
            # loads prefetch during phase 2.
            with tc.tile_pool(name="encp", bufs=1) as encp, \
                 tc.tile_pool(name="wo", bufs=16) as wop, \
                 tc.tile_pool(name="stage", bufs=3) as stp:
                encT = [
                    [encp.tile([128, T], F32R, name=f"encT{n}_{c}",
                               tag=f"encT{n}_{c}") for c in range(2)]
                    for n in range(NHC)
                ]
                with tc.tile_pool(name="pt", bufs=NSC + 2) as ptp, \
                     tc.tile_pool(name="attntmp", bufs=2) as atmp, \
                     tc.tile_pool(name="lpsum", bufs=3, space="PSUM") as lpsum, \
                     tc.tile_pool(name="encpsum", bufs=2, space="PSUM") as epsum, \
                     tc.tile_pool(name="spsum", bufs=1, space="PSUM") as spsum:

                    def normalize(n, j, sumbc, enc_ps):
                        # 1/x as exp(-ln(x)) on the ACT engine: the DVE
                        # InstReciprocal is ~3.3us for 512 columns
                        t0 = j * TT
                        lnm = atmp.tile([128, TT], F32, name="lnm", tag="lnm")
                        nc.scalar.activation(lnm[:], sumbc[:], LN)
                        rrec = atmp.tile([128, TT], F32, name="rrec", tag="rrec")
                        nc.scalar.activation(rrec[:], lnm[:], EXP, scale=-1.0)
                        for c in range(2):
                            nc.vector.tensor_mul(
                                encT[n][c][:, t0 : t0 + TT], enc_ps[c][:], rrec[:]
                            )

                    prev = None
                    for n in range(NHC):
                        for j in range(NTT):
                            t0 = j * TT
                            nsc = (t0 + TT) // SC
                            pts = []

                            def logits_chunk(si):
                                lp = lpsum.tile([128, TT], F32, name="lp", tag="lp")
                                for c in range(2):
                                    nc.tensor.matmul(
                                        lp[:],
                                        KT[c][:, si * SC : (si + 1) * SC],
                                        QT[n][c][:, t0 : t0 + TT],
                                        start=(c == 0),
                                        stop=(c == 1),
                                    )
                                pt = ptp.tile([128, TT], F32R, name="pt", tag="pt")
                                nc.scalar.activation(pt[:], lp[:], EXP)
                                r = si - (TT // SC) * j
                                if r >= 0:
                                    # causal: keep where t - s >= 0, i.e.
                                    # col - part - 128*r >= 0 (on gpsimd:
                                    # 1-input, line-rate, keeps DVE free)
                                    nc.gpsimd.affine_select(
                                        pt[:], pt[:], pattern=[[1, TT]],
                                        compare_op=mybir.AluOpType.is_ge,
                                        fill=0.0, base=-SC * r,
                                        channel_multiplier=-1,
                                    )
                                pts.append(pt)

                            sumbc = spsum.tile([128, TT], F32, name="sumbc",
                                               tag="sumbc")
                            enc_ps = [
                                epsum.tile([128, TT], F32, name=f"enc_ps{c}",
                                           tag=f"enc_ps{c}")
                                for c in range(2)
                            ]

                            def pv_chunk(si):
                                st = si == 0
                                sp = si == nsc - 1
                                nc.tensor.matmul(
                                    sumbc[:], ones_t[:], pts[si][:],
                                    start=st, stop=sp,
                                )
                                for c in range(2):
                                    nc.tensor.matmul(
                                        enc_ps[c][:],
                                        V[si][:, c * 128 : (c + 1) * 128],
                                        pts[si][:],
                                        start=st, stop=sp,
                                    )

                            # software pipeline: logits run two chunks ahead
                            # of the PV/sum consumers; the previous tile's
                            # normalize is slotted after the first logits.
                            logits_chunk(0)
                            if prev is not None:
                                normalize(*prev)
                            logits_chunk(1)
                            for si in range(2, nsc):
                                logits_chunk(si)
                                pv_chunk(si - 2)
                            pv_chunk(nsc - 2)
                            pv_chunk(nsc - 1)
                            prev = (n, j, sumbc, enc_ps)
                    normalize(*prev)

                # ------------- phase 3: output projection -----------------
                with tc.tile_pool(name="opsum", bufs=3, space="PSUM") as opsum:
                    for stream, wo_d, out_d, wdim, st0 in (
                        (0, wo0, out0p, W0, 0),
                        (1, wo1, out1p, W1, S0),
                    ):
                        ntch = (S0 if stream == 0 else S1) // 128
                        for dtp in range(wdim // 1024):
                            wts = {}
                            for n in range(NHC):
                                for c in range(2):
                                    wt = wop.tile([128, 1024], F32R, name="wot",
                                                  tag="wot")
                                    eng = nc.sync if (n + c) % 2 == 0 else nc.gpsimd
                                    eng.dma_start(
                                        wt[:],
                                        wo_d[n, c * 128 : (c + 1) * 128,
                                             dtp * 1024 : (dtp + 1) * 1024],
                                    )
                                    wts[(n, c)] = wt
                            for dt2 in range(2):
                                dcol = dtp * 1024 + dt2 * 512
                                for tch in range(ntch):
                                    tcol = st0 + tch * 128
                                    op = opsum.tile([128, 512], F32, name="op",
                                                    tag="op")
                                    for n in range(NHC):
                                        for c in range(2):
                                            nc.tensor.matmul(
                                                op[:],
                                                encT[n][c][:, tcol : tcol + 128],
                                                wts[(n, c)][:, dt2 * 512 :
                                                            (dt2 + 1) * 512],
                                                start=(n == 0 and c == 0),
                                                stop=(n == NHC - 1 and c == 1),
                                            )
                                    stg = stp.tile([128, 512], F32, name="stg",
                                                   tag="stg")
                                    if tch % 2 == 0:
                                        nc.scalar.copy(stg[:], op[:])
                                    else:
                                        nc.vector.tensor_copy(stg[:], op[:])
                                    eng = nc.sync if tch % 2 == 0 else nc.gpsimd
                                    eng.dma_start(
                                        out_d[tch * 128 : (tch + 1) * 128,
                                              dcol : dcol + 512],
                                        stg[:],
                                    )

    _split_multi_waits(nc)
    return nc


def _host_prep(x0, x1, w_q0, w_kv0, w_out0, w_q1, w_kv1, w_out1, positions,
               attn_mask):
    """Build the 8 per-core input maps."""
    f = np.float32
    freq_exp = (2.0 / HEAD_DIM) * np.arange(HEAD_DIM // 2, dtype=np.float64)
    timescale = MAX_WAVELENGTH ** freq_exp  # [128]

    in_maps = []
    for b in range(B):
        pos = positions[b].astype(np.float64)  # [T]
        rad = pos[None, :] / timescale[:, None]  # [128, T]
        cosk = np.cos(rad).astype(f)
        sink = np.sin(rad).astype(f)
        scale = HEAD_DIM ** -0.5
        cosq = (cosk * scale).astype(f)
        sinq = (sink * scale).astype(f)
        tbls = np.concatenate([cosq, sinq, cosk, sink], axis=1)
        base = {
            "xp0": _pack(x0[b].T.astype(f)),
            "xp1": _pack(x1[b].T.astype(f)),
            "wk0": _pack(w_kv0[0, 0].astype(f)),
            "wv0": _pack(w_kv0[1, 0].astype(f)),
            "wk1": _pack(w_kv1[0, 0].astype(f)),
            "wv1": _pack(w_kv1[1, 0].astype(f)),
            "tbls": np.ascontiguousarray(tbls),
            "onesd": np.ones((128, 128), f),
        }
        for g in range(G):
            hs = slice(g * NHC, (g + 1) * NHC)
            im = dict(base)
            im["wq0"] = np.stack([_pack(w_q0[n].astype(f)) for n in range(*hs.indices(8))])
            im["wq1"] = np.stack([_pack(w_q1[n].astype(f)) for n in range(*hs.indices(8))])
            im["wo0"] = np.ascontiguousarray(w_out0[hs].astype(f))
            im["wo1"] = np.ascontiguousarray(w_out1[hs].astype(f))
            in_maps.append(im)
    return in_maps


def _mask_is_blockwise_causal(attn_mask):
    m = attn_mask[:, 0]  # [B, T, T]
    if m.shape != (B, T, T):
        return False
    tri = np.tril(np.ones((T, T), dtype=bool))
    return bool((m == tri[None]).all())


def _numpy_reference(x0, x1, w_q0, w_kv0, w_out0, w_q1, w_kv1, w_out1,
                     positions, attn_mask):
    """Exact-math fallback (only used if the mask is not causal)."""
    def rope(x, pos):
        d = x.shape[-1]
        fe = (2.0 / d) * np.arange(d // 2, dtype=np.float32)
        ts = MAX_WAVELENGTH ** fe
        rad = pos[..., None].astype(np.float32) / ts
        rad = rad[..., None, :]
        s, c = np.sin(rad), np.cos(rad)
        f, sec = np.split(x.astype(np.float32), 2, axis=-1)
        return np.concatenate([f * c - sec * s, sec * c + f * s], -1)

    q0 = np.einsum("BTD,NDH->BTNH", x0, w_q0)
    k0, v0 = np.einsum("BSD,CKDH->CBSKH", x0, w_kv0)
    q1 = np.einsum("BTD,NDH->BTNH", x1, w_q1)
    k1, v1 = np.einsum("BSD,CKDH->CBSKH", x1, w_kv1)
    q = np.concatenate([q0, q1], 1)
    k = np.concatenate([k0, k1], 1)
    v = np.concatenate([v0, v1], 1)
    q = rope(q, positions) * (HEAD_DIM ** -0.5)
    k = rope(k, positions)
    Bn, Tn = q.shape[0], q.shape[1]
    q = q.reshape(Bn, Tn, 1, NUM_HEADS, HEAD_DIM)
    logits = np.einsum("BTKGH,BSKH->BKGTS", q, k)
    masked = np.where(attn_mask[:, :, None], logits, -2.3819763e38)
    masked = masked - masked.max(-1, keepdims=True)
    e = np.exp(masked)
    probs = e / e.sum(-1, keepdims=True)
    enc = np.einsum("BKGTS,BSKH->BTKGH", probs.astype(np.float32), v)
    enc = enc.reshape(Bn, Tn, NUM_HEADS, HEAD_DIM)
    o0 = np.einsum("BTNH,NHD->BTD", enc[:, :S0], w_out0)
    o1 = np.einsum("BTNH,NHD->BTD", enc[:, S0:], w_out1)
    return (o0.astype(np.float32), o1.astype(np.float32),
            k.astype(np.float32), v.astype(np.float32))


def kernel(x0, x1, w_q0, w_kv0, w_out0, w_q1, w_kv1, w_out1, positions,
           attn_mask):
    if not _mask_is_blockwise_causal(attn_mask):
        return _numpy_reference(x0, x1, w_q0, w_kv0, w_out0, w_q1, w_kv1,
                                w_out1, positions, attn_mask)

    from concourse.bass_utils import run_bass_kernel_spmd

    if "nc" not in _PROGRAM_CACHE:
        _PROGRAM_CACHE["nc"] = _build_program()
    nc = _PROGRAM_CACHE["nc"]

    in_maps = _host_prep(x0, x1, w_q0, w_kv0, w_out0, w_q1, w_kv1, w_out1,
                         positions, attn_mask)
    res = run_bass_kernel_spmd(nc, in_maps, core_ids=list(range(8))).results

    out0 = np.stack(
        [res[2 * b]["out0p"] + res[2 * b + 1]["out0p"] for b in range(B)]
    )
    out1 = np.stack(
        [res[2 * b]["out1p"] + res[2 * b + 1]["out1p"] for b in range(B)]
    )
    k = np.stack(
        [res[2 * b]["kT"].T.reshape(T, 1, HEAD_DIM) for b in range(B)]
    )
    v = np.stack(
        [res[2 * b]["v"].reshape(T, 1, HEAD_DIM) for b in range(B)]
    )
    return out0, out1, k, v


# revision 28
# speedup vs baseline: 1.0146x; 1.0146x over previous
"""Trainium2 Bass kernel for the two-stream GQA attention problem.

Contract: kernel(**inputs) takes the FULL numpy inputs (as produced by
setup_inputs()) and returns the full outputs (out0, out1, k, v), matching
the reference jax implementation.

Distribution: 8 NeuronCores as a 4 (batch) x 2 (head-group) mesh.
Each core handles one batch element and 4 of the 8 query heads; the single
KV head is computed redundantly in each head-group.  The two partial
output projections per batch are summed on the host (the only cross-core
reduction).

Layout: the host pre-packs every streamed tensor into SBUF-shaped
[128, N] partition-major blocks with long contiguous DRAM lines (the DMA
trigger instruction costs ~5-17ns per descriptor line, so few-line
transfers with multi-KB lines are essential).  x is packed transposed
(x^T), after which every matmul consumes natural layouts only:
  Q^T[h,t]   = sum_w w_q[w,h] * x^T[w,t]          (lhsT=w_q tile, rhs=x^T)
  K^T[h,s]   likewise;  V[s,h] = sum_w x^T[w,s]*w_v[w,h] (lhsT=x^T, rhs=w_v)
  L^T[s,t]   = sum_h K^T[h,s] * Q^T[h,t]          (logits, transposed)
  P^T        = exp(L^T), causal-masked in place by a gpsimd
               affine_select on diagonal s-chunks (no max subtraction --
               logits here are O(5), exp cannot overflow, matches softmax)
  enc^T[h,t] = sum_s V[s,h] * P^T[s,t]
  sumbc[:,t] = sum_s P^T[s,t]  (all-ones [128,128] stationary: the column
               sums land broadcast across all 128 partitions, so the
               reciprocal runs on all DVE lanes)
  out[t,d]   = sum_{n,h} (enc^T[h,t]/sumbc[t]) * w_out[h,d]
RoPE is applied with DVE elementwise ops against host-precomputed
sin/cos tables laid out as [h_dim, t]; the 1/sqrt(HEAD_DIM) query scale is
folded into the q tables.  Causality is exploited by skipping fully-masked
s-chunks entirely and zeroing the diagonal chunks with affine_select.

All matmuls use float32r (full-rate fp32 PE path).  Outputs and constants
ride the gpsimd (SWDGE) queue so they never block the input stream on the
sync (HWDGE) queue.
"""

import numpy as np

HEAD_DIM = 256
NUM_HEADS = 8
MAX_WAVELENGTH = 10000.0

B, S0, S1 = 4, 768, 256
W0, W1 = 2048, 1024
T = S0 + S1  # 1024
NHC = 4      # heads per core
G = 2        # head groups
TT = 512     # attention t-tile
NTT = T // TT
SC = 128     # s-chunk
NSC = T // SC

_PROGRAM_CACHE = {}


# ----------------------------------------------------------------------------
# Workaround: this walrus build rejects instructions carrying more than one
# sync-wait command.  Split every multi-wait instruction into same-engine
# single-wait nops inserted right before it (engines run their stream in
# order, so semantics are identical).
# ----------------------------------------------------------------------------
def _split_multi_waits(nc, max_waits=1):
    import concourse.mybir as mybir

    for f in nc.m.functions:
        for bb in f.blocks:
            insts = bb.instructions
            i = 0
            while i < len(insts):
                inst = insts[i]
                si = getattr(inst, "sync_info", None)
                waits = list(si.on_wait) if si is not None and si.on_wait else []
                if len(waits) > max_waits:
                    si.on_wait = waits[:max_waits]
                    rest = waits[max_waits:]
                    eng = nc.engines[inst.engine]
                    for j in range(0, len(rest), max_waits):
                        chunk = rest[j : j + max_waits]
                        bi = eng.nop()
                        nop_inst = bi.ins
                        src = nc.cur_bb.bb.instructions
                        assert src[-1] is nop_inst
                        src.pop()
                        nsi = nop_inst.sync_info
                        if nsi is None:
                            nop_inst.sync_info = mybir.SyncInfo(
                                on_wait=chunk, on_update=[]
                            )
                        else:
                            nsi.on_wait = chunk
                        insts.insert(i, nop_inst)
                        i += 1
                i += 1


def _pack(mat):
    """[nw*128, H] -> [128, nw*H] partition-major (row r of tile i at
    partition r, columns i*H..)."""
    nw = mat.shape[0] // 128
    return np.ascontiguousarray(
        mat.reshape(nw, 128, -1).transpose(1, 0, 2).reshape(128, -1)
    )


def _build_program():
    import concourse.bass as bass
    import concourse.mybir as mybir
    import concourse.tile as tile
    import concourse.tile_utils as tile_utils

    # stale 192KB cap leaves 16KB/partition unused on trn2 (208KB usable)
    if getattr(tile_utils, "max_sbuf_usage", 0) < 204 * 1024:
        tile_utils.max_sbuf_usage = 204 * 1024

    F32 = mybir.dt.float32
    F32R = mybir.dt.float32r
    EXP = mybir.ActivationFunctionType.Exp
    LN = mybir.ActivationFunctionType.Ln

    nc = bass.Bass("TRN2", target_bir_lowering=False, debug=False)

    def din(name, shape, dt=F32R):
        return nc.dram_tensor(name, list(shape), dt, kind="ExternalInput").ap()

    # all inputs pre-packed to [128, N] partition-major on the host
    xp0 = din("xp0", [128, 16 * S0])
    xp1 = din("xp1", [128, 8 * S1])
    wq0 = din("wq0", [NHC, 128, 16 * HEAD_DIM])
    wq1 = din("wq1", [NHC, 128, 8 * HEAD_DIM])
    wk0 = din("wk0", [128, 16 * HEAD_DIM])
    wk1 = din("wk1", [128, 8 * HEAD_DIM])
    wv0 = din("wv0", [128, 16 * HEAD_DIM])
    wv1 = din("wv1", [128, 8 * HEAD_DIM])
    wo0 = din("wo0", [NHC, HEAD_DIM, W0])
    wo1 = din("wo1", [NHC, HEAD_DIM, W1])
    # cosq | sinq | cosk | sink concatenated along the free axis
    tbls = din("tbls", [128, 4 * T], F32)
    onesd = din("onesd", [128, 128])

    out0p = nc.dram_tensor("out0p", [S0, W0], F32, kind="ExternalOutput").ap()
    out1p = nc.dram_tensor("out1p", [S1, W1], F32, kind="ExternalOutput").ap()
    kT_d = nc.dram_tensor("kT", [HEAD_DIM, T], F32R, kind="ExternalOutput").ap()
    v_d = nc.dram_tensor("v", [T, HEAD_DIM], F32R, kind="ExternalOutput").ap()

    # token segments: (t0, length, stream); stream 0 -> xp0, stream 1 -> xp1
    SEGS = [(0, 512, 0), (512, 256, 0), (768, 256, 1)]
    NW = {0: W0 // 128, 1: W1 // 128}

    import contextlib

    with tile.TileContext(nc) as tc, \
         nc.allow_low_precision("float32r is a rounded fp32 for the PE fast path"):
        with contextlib.ExitStack() as ctx:
            persist = ctx.enter_context(tc.tile_pool(name="persist", bufs=1))

            # persistent activation tensors (encT lives in a later scope)
            QT = [
                [persist.tile([128, T], F32R, name=f"QT{n}_{c}", tag=f"QT{n}_{c}")
                 for c in range(2)]
                for n in range(NHC)
            ]
            KT = [persist.tile([128, T], F32R, name=f"KT{c}", tag=f"KT{c}")
                  for c in range(2)]
            V = [persist.tile([128, HEAD_DIM], F32R, name=f"V{s}", tag=f"V{s}")
                 for s in range(NSC)]

            # constants (DMAs are emitted inside phase 1, after the
            # startup-critical weight loads, on the gpsimd/SWDGE queue)
            ttbl = persist.tile([128, 4 * T], F32, name="ttbl", tag="ttbl")
            tcosq = ttbl[:, 0 * T : 1 * T]
            tsinq = ttbl[:, 1 * T : 2 * T]
            tcosk = ttbl[:, 2 * T : 3 * T]
            tsink = ttbl[:, 3 * T : 4 * T]
            ones_t = persist.tile([128, 128], F32R, name="ones_t", tag="ones_t")

            # ---------------- phase 1: projections -----------------------
            def rope_muls(qp0, qp1, cos_t, sin_t, t0, ln, tmp_pool):
                # DVE multiplies (psum-reading); qp0 reads first so its psum
                # bank frees early.  The add/sub combines run on gpsimd.
                cs = cos_t[:, t0 : t0 + ln]
                sn = sin_t[:, t0 : t0 + ln]
                a = tmp_pool.tile([128, 512], F32, name="rope_a", tag="rope_a")
                nc.vector.tensor_mul(a[:, :ln], qp0[:], cs)
                b2 = tmp_pool.tile([128, 512], F32, name="rope_b2", tag="rope_b2")
                nc.vector.tensor_mul(b2[:, :ln], qp0[:], sn)
                b = tmp_pool.tile([128, 512], F32, name="rope_b", tag="rope_b")
                nc.vector.tensor_mul(b[:, :ln], qp1[:], sn)
                a2 = tmp_pool.tile([128, 512], F32, name="rope_a2", tag="rope_a2")
                nc.vector.tensor_mul(a2[:, :ln], qp1[:], cs)
                return a, b, a2, b2

            def rope_combine(parts, dst0, dst1, t0, ln):
                a, b, a2, b2 = parts
                nc.gpsimd.tensor_sub(dst0[:, t0 : t0 + ln], a[:, :ln], b[:, :ln])
                nc.gpsimd.tensor_add(dst1[:, t0 : t0 + ln], a2[:, :ln], b2[:, :ln])

            def load_packed(src, ncols, nsplit, wpool, tag, eng=None):
                """Load a packed [128, ncols] DRAM tensor in nsplit 2D DMAs;
                returns the list of SBUF tiles (each [128, ncols/nsplit])."""
                step = ncols // nsplit
                out = []
                for bi in range(nsplit):
                    t = wpool.tile([128, step], F32R, name=tag, tag=tag)
                    (eng or nc.sync).dma_start(
                        t[:], src[:, bi * step : (bi + 1) * step]
                    )
                    out.append(t)
                return out

            def qk_proj(w0_sl, w1_sl, cos_t, sin_t, dst, qpsum, tmp_pool,
                        pset=0):
                # psum tags: A{c} [512] / B{c} [256]; two alternating tag sets
                # so consecutive heads never wait on each other's ropes.
                for stream, w_sl in ((0, w0_sl), (1, w1_sl)):
                    segs = [s for s in SEGS if s[2] == stream]
                    ps = {}
                    for c in range(2):
                        for (t0, ln, _s) in segs:
                            tag = (f"qk_psA{c}_{pset}" if ln == 512 else
                                   f"qk_psB{c}_{pset}")
                            ps[(c, t0)] = qpsum.tile([128, ln], F32, name=tag,
                                                     tag=tag)
                    nw = NW[stream]
                    xt = xt0 if stream == 0 else xt1
                    for i in range(nw):
                        wt = w_sl(i)
                        for c in range(2):
                            lhs = wt[:, c * 128 : (c + 1) * 128]
                            for (t0, ln, _s) in segs:
                                rel0 = t0 if stream == 0 else t0 - S0
                                nc.tensor.matmul(
                                    ps[(c, t0)][:],
                                    lhs,
                                    xt[i](rel0, ln),
                                    start=(i == 0),
                                    stop=(i == nw - 1),
                                )
                    parts = {}
                    for (t0, ln, _s) in segs:
                        parts[t0] = rope_muls(ps[(0, t0)], ps[(1, t0)],
                                              cos_t, sin_t, t0, ln, tmp_pool)
                    for (t0, ln, _s) in segs:
                        rope_combine(parts[t0], dst[0], dst[1], t0, ln)

            with tc.tile_pool(name="xt", bufs=1) as xtp, \
                 tc.tile_pool(name="wtiles", bufs=6) as wpool, \
                 tc.tile_pool(name="ropetmp", bufs=3) as tmp_pool:
                # startup-ordered loads: first K weights and the x batches
                # they pair with, interleaved on the sync queue; wk1 and the
                # constants ride the gpsimd queue in parallel.
                wk0b = []
                wk0b.append(wpool.tile([128, 8 * HEAD_DIM], F32R, name="wb",
                                       tag="wb"))
                nc.sync.dma_start(wk0b[0][:], wk0[:, : 8 * HEAD_DIM])
                xtb0 = []

                def _load_x0(bi):
                    t = xtp.tile([128, 2 * S0], F32R, name=f"xtb0_{bi}",
                                 tag=f"xtb0_{bi}")
                    nc.sync.dma_start(
                        t[:], xp0[:, bi * 2 * S0 : (bi + 1) * 2 * S0]
                    )
                    xtb0.append(t)

                for bi in range(3):
                    _load_x0(bi)
                wk0b.append(wpool.tile([128, 8 * HEAD_DIM], F32R, name="wb",
                                       tag="wb"))
                nc.sync.dma_start(wk0b[1][:], wk0[:, 8 * HEAD_DIM :])
                for bi in range(3, 8):
                    _load_x0(bi)
                xtb1 = xtp.tile([128, 8 * S1], F32R, name="xtb1", tag="xtb1")
                nc.sync.dma_start(xtb1[:], xp1[:])
                nc.gpsimd.dma_start(ones_t[:], onesd[:])
                wk1b = wpool.tile([128, 8 * HEAD_DIM], F32R, name="wb", tag="wb")
                nc.gpsimd.dma_start(wk1b[:], wk1[:])
                nc.gpsimd.dma_start(ttbl[:], tbls[:])

                # HAM warmup: dummy matmuls keep the PE clock at 8/8 while the
                # initial DMA stream lands (PE has nothing real to do yet).
                with tc.tile_pool(name="warm", bufs=1, space="PSUM") as warmp:
                    wps = warmp.tile([128, 128], F32, name="warm_ps",
                                     tag="warm_ps")
                    for wi in range(60):
                        nc.tensor.matmul(wps[:], ones_t[:], ones_t[:],
                                         start=(wi % 10 == 0),
                                         stop=(wi % 10 == 9))

                def _xt0(i):
                    base = (i % 2) * S0
                    return lambda rel0, ln: xtb0[i // 2][:, base + rel0 :
                                                         base + rel0 + ln]

                def _xt1(i):
                    base = (i % 8) * S1
                    return lambda rel0, ln: xtb1[:, base + rel0 : base + rel0 + ln]

                xt0 = [_xt0(i) for i in range(NW[0])]
                xt1 = [_xt1(i) for i in range(NW[1])]

                # K projection + rope + output
                with tc.tile_pool(name="qkpsum", bufs=1, space="PSUM") as qpsum:
                    qk_proj(
                        lambda i: wk0b[i // 8][:, (i % 8) * HEAD_DIM :
                                               (i % 8 + 1) * HEAD_DIM],
                        lambda i: wk1b[:, i * HEAD_DIM : (i + 1) * HEAD_DIM],
                        tcosk, tsink, KT, qpsum, tmp_pool, pset=0)
                for c in range(2):
                    nc.gpsimd.dma_start(kT_d[c * 128 : (c + 1) * 128, :], KT[c][:])

                # V projection (natural layout), all 6+2 chunks in one pass
                with tc.tile_pool(name="vpsum", bufs=1, space="PSUM") as vpsum:
                    for stream, wv_d in ((0, wv0), (1, wv1)):
                        nw = NW[stream]
                        xt = xt0 if stream == 0 else xt1
                        scs = range(0, 6) if stream == 0 else range(6, 8)
                        vps = {
                            sc: vpsum.tile([128, HEAD_DIM], F32,
                                           name=f"v_ps{sc % 6}", tag=f"v_ps{sc % 6}")
                            for sc in scs
                        }
                        wb = load_packed(wv_d, nw * HEAD_DIM, max(1, nw // 8),
                                         wpool, "wb")
                        for i in range(nw):
                            wt = wb[i // 8][:, (i % 8) * HEAD_DIM :
                                            (i % 8) * HEAD_DIM + HEAD_DIM]
                            for sc in scs:
                                rel0 = sc * 128 if stream == 0 else sc * 128 - S0
                                nc.tensor.matmul(
                                    vps[sc][:],
                                    xt[i](rel0, 128),
                                    wt[:],
                                    start=(i == 0),
                                    stop=(i == nw - 1),
                                )
                        for sc in scs:
                            nc.scalar.copy(V[sc][:], vps[sc][:])
                            nc.gpsimd.dma_start(
                                v_d[sc * 128 : (sc + 1) * 128, :], V[sc][:]
                            )

                # Q projections
                with tc.tile_pool(name="qpsum", bufs=1, space="PSUM") as qpsum:
                    for n in range(NHC):
                        wb0 = load_packed(wq0[n], 16 * HEAD_DIM, 2, wpool, "wb")
                        wb1 = load_packed(wq1[n], 8 * HEAD_DIM, 1, wpool, "wb")

                        def _slb(wb):
                            return lambda i: wb[i // 8][
                                :, (i % 8) * HEAD_DIM : (i % 8 + 1) * HEAD_DIM
                            ]

                        qk_proj(_slb(wb0), _slb(wb1), tcosq, tsinq, QT[n],
                                qpsum, tmp_pool, pset=(n + 1) % 2)

            # ---------------- phases 2+3 ----------------------------------
            # encT/wo/stage pools open BEFORE the attention pools so their
            # SBUF region is disjoint from the attention tiles: the wo DMA
            # loads prefetch during phase 2.
            with tc.tile_pool(name="encp", bufs=1) as encp, \
                 tc.tile_pool(name="wo", bufs=16) as wop, \
                 tc.tile_pool(name="stage", bufs=3) as stp:
                encT = [
                    [encp.tile([128, T], F32R, name=f"encT{n}_{c}",
                               tag=f"encT{n}_{c}") for c in range(2)]
                    for n in range(NHC)
                ]
                with tc.tile_pool(name="pt", bufs=NSC + 2) as ptp, \
                     tc.tile_pool(name="attntmp", bufs=2) as atmp, \
                     tc.tile_pool(name="lpsum", bufs=3, space="PSUM") as lpsum, \
                     tc.tile_pool(name="encpsum", bufs=2, space="PSUM") as epsum, \
                     tc.tile_pool(name="spsum", bufs=1, space="PSUM") as spsum:

                    def normalize(n, j, sumbc, enc_ps):
                        # 1/x as exp(-ln(x)) on the ACT engine: the DVE
                        # InstReciprocal is ~3.3us for 512 columns
                        t0 = j * TT
                        lnm = atmp.tile([128, TT], F32, name="lnm", tag="lnm")
                        nc.scalar.activation(lnm[:], sumbc[:], LN)
                        rrec = atmp.tile([128, TT], F32, name="rrec", tag="rrec")
                        nc.scalar.activation(rrec[:], lnm[:], EXP, scale=-1.0)
                        for c in range(2):
                            nc.vector.tensor_mul(
                                encT[n][c][:, t0 : t0 + TT], enc_ps[c][:], rrec[:]
                            )

                    prev = None
                    for n in range(NHC):
                        for j in range(NTT):
                            t0 = j * TT
                            nsc = (t0 + TT) // SC
                            pts = []

                            def logits_chunk(si):
                                lp = lpsum.tile([128, TT], F32, name="lp", tag="lp")
                                for c in range(2):
                                    nc.tensor.matmul(
                                        lp[:],
                                        KT[c][:, si * SC : (si + 1) * SC],
                                        QT[n][c][:, t0 : t0 + TT],
                                        start=(c == 0),
                                        stop=(c == 1),
                                    )
                                pt = ptp.tile([128, TT], F32R, name="pt", tag="pt")
                                nc.scalar.activation(pt[:], lp[:], EXP)
                                r = si - (TT // SC) * j
                                if r >= 0:
                                    # causal: keep where t - s >= 0, i.e.
                                    # col - part - 128*r >= 0 (on gpsimd:
                                    # 1-input, line-rate, keeps DVE free)
                                    nc.gpsimd.affine_select(
                                        pt[:], pt[:], pattern=[[1, TT]],
                                        compare_op=mybir.AluOpType.is_ge,
                                        fill=0.0, base=-SC * r,
                                        channel_multiplier=-1,
                                    )
                                pts.append(pt)

                            sumbc = spsum.tile([128, TT], F32, name="sumbc",
                                               tag="sumbc")
                            enc_ps = [
                                epsum.tile([128, TT], F32, name=f"enc_ps{c}",
                                           tag=f"enc_ps{c}")
                                for c in range(2)
                            ]

                            def pv_chunk(si):
                                st = si == 0
                                sp = si == nsc - 1
                                nc.tensor.matmul(
                                    sumbc[:], ones_t[:], pts[si][:],
                                    start=st, stop=sp,
                                )
                                for c in range(2):
                                    nc.tensor.matmul(
                                        enc_ps[c][:],
                                        V[si][:, c * 128 : (c + 1) * 128],
                                        pts[si][:],
                                        start=st, stop=sp,
                                    )

                            # software pipeline: logits run two chunks ahead
                            # of the PV/sum consumers; the previous tile's
                            # normalize is slotted after the first logits.
                            logits_chunk(0)
                            if prev is not None:
                                normalize(*prev)
                            logits_chunk(1)
                            for si in range(2, nsc):
                                logits_chunk(si)
                                pv_chunk(si - 2)
                            pv_chunk(nsc - 2)
                            pv_chunk(nsc - 1)
                            prev = (n, j, sumbc, enc_ps)
                    normalize(*prev)

                # ------------- phase 3: output projection -----------------
                with tc.tile_pool(name="opsum", bufs=3, space="PSUM") as opsum:
                    for stream, wo_d, out_d, wdim, st0 in (
                        (0, wo0, out0p, W0, 0),
                        (1, wo1, out1p, W1, S0),
                    ):
                        ntch = (S0 if stream == 0 else S1) // 128
                        for dtp in range(wdim // 1024):
                            wts = {}
                            for n in range(NHC):
                                for c in range(2):
                                    wt = wop.tile([128, 1024], F32R, name="wot",
                                                  tag="wot")
                                    eng = nc.sync if (n + c) % 2 == 0 else nc.gpsimd
                                    eng.dma_start(
                                        wt[:],
                                        wo_d[n, c * 128 : (c + 1) * 128,
                                             dtp * 1024 : (dtp + 1) * 1024],
                                    )
                                    wts[(n, c)] = wt
                            for dt2 in range(2):
                                dcol = dtp * 1024 + dt2 * 512
                                for tch in range(ntch):
                                    tcol = st0 + tch * 128
                                    op = opsum.tile([128, 512], F32, name="op",
                                                    tag="op")
                                    for n in range(NHC):
                                        for c in range(2):
                                            nc.tensor.matmul(
                                                op[:],
                                                encT[n][c][:, tcol : tcol + 128],
                                                wts[(n, c)][:, dt2 * 512 :
                                                            (dt2 + 1) * 512],
                                                start=(n == 0 and c == 0),
                                                stop=(n == NHC - 1 and c == 1),
                                            )
                                    stg = stp.tile([128, 512], F32, name="stg",
                                                   tag="stg")
                                    if tch % 2 == 0:
                                        nc.scalar.copy(stg[:], op[:])
                                    else:
                                        nc.vector.tensor_copy(stg[:], op[:])
                                    eng = nc.sync if tch % 2 == 0 else nc.gpsimd
                                    eng.dma_start(
                                        out_d[tch * 128 : (tch + 1) * 128,
                                              dcol : dcol + 512],
                                        stg[:],
                                    )

    _split_multi_waits(nc)
    return nc


def _host_prep(x0, x1, w_q0, w_kv0, w_out0, w_q1, w_kv1, w_out1, positions,
               attn_mask):
    """Build the 8 per-core input maps."""
    f = np.float32
    freq_exp = (2.0 / HEAD_DIM) * np.arange(HEAD_DIM // 2, dtype=np.float64)
    timescale = MAX_WAVELENGTH ** freq_exp  # [128]

    in_maps = []
    for b in range(B):
        pos = positions[b].astype(np.float64)  # [T]
        rad = pos[None, :] / timescale[:, None]  # [128, T]
        cosk = np.cos(rad).astype(f)
        sink = np.sin(rad).astype(f)
        scale = HEAD_DIM ** -0.5
        cosq = (cosk * scale).astype(f)
        sinq = (sink * scale).astype(f)
        tbls = np.concatenate([cosq, sinq, cosk, sink], axis=1)
        base = {
            "xp0": _pack(x0[b].T.astype(f)),
            "xp1": _pack(x1[b].T.astype(f)),
            "wk0": _pack(w_kv0[0, 0].astype(f)),
            "wv0": _pack(w_kv0[1, 0].astype(f)),
            "wk1": _pack(w_kv1[0, 0].astype(f)),
            "wv1": _pack(w_kv1[1, 0].astype(f)),
            "tbls": np.ascontiguousarray(tbls),
            "onesd": np.ones((128, 128), f),
        }
        for g in range(G):
            hs = slice(g * NHC, (g + 1) * NHC)
            im = dict(base)
            im["wq0"] = np.stack([_pack(w_q0[n].astype(f)) for n in range(*hs.indices(8))])
            im["wq1"] = np.stack([_pack(w_q1[n].astype(f)) for n in range(*hs.indices(8))])
            im["wo0"] = np.ascontiguousarray(w_out0[hs].astype(f))
            im["wo1"] = np.ascontiguousarray(w_out1[hs].astype(f))
            in_maps.append(im)
    return in_maps


def _mask_is_blockwise_causal(attn_mask):
    m = attn_mask[:, 0]  # [B, T, T]
    if m.shape != (B, T, T):
        return False
    tri = np.tril(np.ones((T, T), dtype=bool))
    return bool((m == tri[None]).all())


def _numpy_reference(x0, x1, w_q0, w_kv0, w_out0, w_q1, w_kv1, w_out1,
                     positions, attn_mask):
    """Exact-math fallback (only used if the mask is not causal)."""
    def rope(x, pos):
        d = x.shape[-1]
        fe = (2.0 / d) * np.arange(d // 2, dtype=np.float32)
        ts = MAX_WAVELENGTH ** fe
        rad = pos[..., None].astype(np.float32) / ts
        rad = rad[..., None, :]
        s, c = np.sin(rad), np.cos(rad)
        f, sec = np.split(x.astype(np.float32), 2, axis=-1)
        return np.concatenate([f * c - sec * s, sec * c + f * s], -1)

    q0 = np.einsum("BTD,NDH->BTNH", x0, w_q0)
    k0, v0 = np.einsum("BSD,CKDH->CBSKH", x0, w_kv0)
    q1 = np.einsum("BTD,NDH->BTNH", x1, w_q1)
    k1, v1 = np.einsum("BSD,CKDH->CBSKH", x1, w_kv1)
    q = np.concatenate([q0, q1], 1)
    k = np.concatenate([k0, k1], 1)
    v = np.concatenate([v0, v1], 1)
    q = rope(q, positions) * (HEAD_DIM ** -0.5)
    k = rope(k, positions)
    Bn, Tn = q.shape[0], q.shape[1]
    q = q.reshape(Bn, Tn, 1, NUM_HEADS, HEAD_DIM)
    logits = np.einsum("BTKGH,BSKH->BKGTS", q, k)
    masked = np.where(attn_mask[:, :, None], logits, -2.3819763e38)
    masked = masked - masked.max(-1, keepdims=True)
    e = np.exp(masked)
    probs = e / e.sum(-1, keepdims=True)
    enc = np.einsum("BKGTS,BSKH->BTKGH", probs.astype(np.float32), v)
    enc = enc.reshape(Bn, Tn, NUM_HEADS, HEAD_DIM)
    o0 = np.einsum("BTNH,NHD->BTD", enc[:, :S0], w_out0)
    o1 = np.einsum("BTNH,NHD->BTD", enc[:, S0:], w_out1)
    return (o0.astype(np.float32), o1.astype(np.float32),
            k.astype(np.float32), v.astype(np.float32))


def kernel(x0, x1, w_q0, w_kv0, w_out0, w_q1, w_kv1, w_out1, positions,
           attn_mask):
    if not _mask_is_blockwise_causal(attn_mask):
        return _numpy_reference(x0, x1, w_q0, w_kv0, w_out0, w_q1, w_kv1,
                                w_out1, positions, attn_mask)

    from concourse.bass_utils import run_bass_kernel_spmd

    if "nc" not in _PROGRAM_CACHE:
        _PROGRAM_CACHE["nc"] = _build_program()
    nc = _PROGRAM_CACHE["nc"]

    in_maps = _host_prep(x0, x1, w_q0, w_kv0, w_out0, w_q1, w_kv1, w_out1,
                         positions, attn_mask)
    res = run_bass_kernel_spmd(nc, in_maps, core_ids=list(range(8))).results

    out0 = np.stack(
        [res[2 * b]["out0p"] + res[2 * b + 1]["out0p"] for b in range(B)]
    )
    out1 = np.stack(
        [res[2 * b]["out1p"] + res[2 * b + 1]["out1p"] for b in range(B)]
    )
    k = np.stack(
        [res[2 * b]["kT"].T.reshape(T, 1, HEAD_DIM) for b in range(B)]
    )
    v = np.stack(
        [res[2 * b]["v"].reshape(T, 1, HEAD_DIM) for b in range(B)]
    )
    return out0, out1, k, v
